# revision 3
# baseline (speedup 1.0000x reference)
"""Trainium2 Bass kernel v2 for nn_CNNModel_42064909697048.

Strategy: f32-ordered sort keys with the group-local element index embedded
in the low 5 mantissa bits (value & ~31 | idx). One split sorting network per
axis (two Batcher-14 sorts + bitonic pairing) yields min/argmin, max/argmax,
median/argmedian simultaneously; means via windowed reduces; 4-layer MLP +
softmax on PE/ACT. Axis-1 network runs on DVE, axis-2 on GpSimd, in parallel.

Data parallel over 8 NeuronCores; self-contained (hardcoded shapes).
"""

import numpy as np

import concourse.bass as bass
import concourse.mybir as mybir
import concourse.tile as tile_mod
from concourse.tile import TileContext
from concourse.bass_utils import run_bass_kernel_spmd
from concourse.alu_op_type import AluOpType

# ---------------------------------------------------------------- constants
B_TOTAL = 131072
N_CORES = 8
B_CORE = B_TOTAL // N_CORES          # 16384
H = 28
D = 784
P = 128
N_TILES = B_CORE // P                # 128
T = 16                               # tiles per batch
FP = H * T                           # 448 slots per plane
NF = 392
F32 = mybir.dt.float32
I32 = mybir.dt.int32
AXX = mybir.AxisListType.X
MIN = AluOpType.min
MAX = AluOpType.max
ADD = AluOpType.add
IDX_BASE = 0x4B000000                # f32 bits of 8388608.0; |idx -> 2^23+idx

BLOCKS = ["min_v1", "min_i1", "min_v2", "min_i2",
          "max_v1", "max_i1", "max_v2", "max_i2",
          "mean_1", "mean_2",
          "med_v1", "med_i1", "med_v2", "med_i2"]
BOFF = {k: i * H for i, k in enumerate(BLOCKS)}


def _batcher(n):
    m = 1
    while m < n:
        m *= 2
    net = []

    def merge(lo, cnt, r):
        step = r * 2
        if step < cnt:
            merge(lo, cnt, step)
            merge(lo + r, cnt, step)
            for i in range(lo + r, lo + cnt - r, step):
                net.append((i, i + r))
        else:
            net.append((lo, lo + r))

    def sort(lo, cnt):
        if cnt > 1:
            h2 = cnt // 2
            sort(lo, h2)
            sort(lo + h2, h2)
            merge(lo, cnt, 1)

    sort(0, m)
    return [(a, b) for (a, b) in net if b < n]


NET14 = _batcher(14)                 # 53 compare-exchanges


def _levels(net, n=14):
    """Group comparators into dependency levels for stall-free emission."""
    lvl_of = [0] * n
    levels = []
    for (i, j) in net:
        L = max(lvl_of[i], lvl_of[j])
        if L == len(levels):
            levels.append([])
        levels[L].append((i, j))
        lvl_of[i] = lvl_of[j] = L + 1
    return levels


NET14_LEVELS = _levels(NET14)
NSCR = max(len(l) for l in NET14_LEVELS)   # max CEs per level

# ------------------------------------------------- tile tail-drain workaround
def _patched_drain_and_barrier(self, tick_clock, wait_clock):
    drain_inst = self.nc.sync.drain()
    wait_clock.add_sem_waits(
        drain_inst.ins, tile_mod.ScopedClock({None: tick_clock.global_clock})
    )
    si = drain_inst.ins.sync_info
    waits = list(si.on_wait or [])
    if len(waits) > 1:
        si.on_wait = waits[:1]
        for w in waits[1:]:
            d2 = self.nc.sync.drain()
            si2 = d2.ins.sync_info
            if si2 is None:
                d2.ins.sync_info = mybir.SyncInfo(on_wait=[w], on_update=[])
            else:
                si2.on_wait = [w]
    self.nc.all_engine_barrier()
    assert self.sems is not None
    popped = self.nc._tile_sem_poison_stack.pop()
    assert popped is self._sem_poison
    self.nc.clear_and_free_semaphores(list(self.sems.allocated().values()))
    self.nc.all_engine_barrier()


tile_mod.TileContext._drain_and_barrier = _patched_drain_and_barrier

MAX_WAITS = 1


def _split_excess_waits(nc):
    """Walrus in this container rejects instructions with >MAX_WAITS sem
    waits; hoist the excess onto NoOp carriers inserted just before."""
    import bass_rust
    ctr = [0]
    for f in nc.m.functions:
        for blk in f.blocks:
            insts = list(blk.instructions)
            out = []
            changed = False
            for inst in insts:
                si = inst.sync_info
                waits = list(si.on_wait) if (si and si.on_wait) else []
                if len(waits) > MAX_WAITS:
                    changed = True
                    excess = waits[:-MAX_WAITS]
                    si.on_wait = waits[-MAX_WAITS:]
                    for k in range(0, len(excess), MAX_WAITS):
                        nop = bass_rust.InstNoOp(
                            name=f"WSPLIT-{ctr[0]}", ins=[], outs=[])
                        ctr[0] += 1
                        nop.engine = inst.engine
                        nop.sync_info = mybir.SyncInfo(
                            on_wait=excess[k:k + MAX_WAITS], on_update=[])
                        out.append(nop)
                out.append(inst)
            if changed:
                blk.instructions = out


# ------------------------------------------------------------- bass program
def build_nc(n_tiles: int = N_TILES, mm_dtype: str = "f32r",
             debug_features: bool = False, reps: int = 1):
    nb = n_tiles // T
    nc = bass.Bass()
    t_in = nc.dram_tensor("t", [P * n_tiles, D], F32, kind="ExternalInput")
    w1 = nc.dram_tensor("w1", [NF, 270], F32, kind="ExternalInput")
    b1 = nc.dram_tensor("b1", [270, 1], F32, kind="ExternalInput")
    w2 = nc.dram_tensor("w2", [270, 90], F32, kind="ExternalInput")
    b2 = nc.dram_tensor("b2", [90, 1], F32, kind="ExternalInput")
    w3 = nc.dram_tensor("w3", [90, 30], F32, kind="ExternalInput")
    b3 = nc.dram_tensor("b3", [30, 1], F32, kind="ExternalInput")
    w4 = nc.dram_tensor("w4", [30, 10], F32, kind="ExternalInput")
    b4 = nc.dram_tensor("b4", [10, 1], F32, kind="ExternalInput")
    idn = nc.dram_tensor("idn", [P, P], F32, kind="ExternalInput")
    if debug_features:
        y_out = nc.dram_tensor("y", [P * n_tiles, NF], F32,
                               kind="ExternalOutput")
    else:
        y_out = nc.dram_tensor("y", [P * n_tiles, 16], F32,
                               kind="ExternalOutput")

    MMDT = {"f32": F32, "f32r": mybir.dt.float32r}[mm_dtype]

    def mm(ps, lhs, rhs, start, stop):
        if MMDT is F32:
            nc.tensor.matmul(ps, lhs, rhs, start=start, stop=stop)
        else:
            nc.tensor.matmul(ps, lhs.bitcast(MMDT), rhs.bitcast(MMDT),
                             start=start, stop=stop)

    with TileContext(nc) as tc:
        with (
            tc.tile_pool(name="wpool", bufs=1) as wpool,
            tc.tile_pool(name="xpool", bufs=1) as xpool,
            tc.tile_pool(name="ppool", bufs=1) as ppool,
            tc.tile_pool(name="fpool", bufs=1) as fpool,
            tc.tile_pool(name="gpool", bufs=1) as gpool,
            tc.tile_pool(name="mpool", bufs=2) as mpool,
            tc.tile_pool(name="psA", bufs=2, space="PSUM") as psA,
            tc.tile_pool(name="psB", bufs=1, space="PSUM") as psB,
            tc.tile_pool(name="psC", bufs=1, space="PSUM") as psC,
        ):
            # ---- static weights into SBUF
            w1_t = [wpool.tile([128, 270], F32, name=f"w1_{i}", tag=f"w1_{i}")
                    for i in range(3)]
            w1_t.append(wpool.tile([8, 270], F32, name="w1_3", tag="w1_3"))
            for i in range(3):
                nc.sync.dma_start(w1_t[i][:], w1[128 * i:128 * (i + 1), :])
            nc.sync.dma_start(w1_t[3][:], w1[384:392, :])
            w2_t = [wpool.tile([128, 90], F32, name="w2_0", tag="w2_0"),
                    wpool.tile([128, 90], F32, name="w2_1", tag="w2_1"),
                    wpool.tile([14, 90], F32, name="w2_2", tag="w2_2")]
            nc.sync.dma_start(w2_t[0][:], w2[0:128, :])
            nc.sync.dma_start(w2_t[1][:], w2[128:256, :])
            nc.sync.dma_start(w2_t[2][:], w2[256:270, :])
            w3_t = wpool.tile([90, 30], F32, name="w3", tag="w3")
            nc.sync.dma_start(w3_t[:], w3[:, :])
            w4_t = wpool.tile([30, 10], F32, name="w4", tag="w4")
            nc.sync.dma_start(w4_t[:], w4[:, :])
            b1_t = [wpool.tile([128, 1], F32, name="b1_0", tag="b1_0"),
                    wpool.tile([128, 1], F32, name="b1_1", tag="b1_1"),
                    wpool.tile([14, 1], F32, name="b1_2", tag="b1_2")]
            nc.sync.dma_start(b1_t[0][:], b1[0:128, :])
            nc.sync.dma_start(b1_t[1][:], b1[128:256, :])
            nc.sync.dma_start(b1_t[2][:], b1[256:270, :])
            b2_t = wpool.tile([90, 1], F32, name="b2", tag="b2")
            nc.sync.dma_start(b2_t[:], b2[:, :])
            b3_t = wpool.tile([30, 1], F32, name="b3", tag="b3")
            nc.sync.dma_start(b3_t[:], b3[:, :])
            b4_t = wpool.tile([10, 1], F32, name="b4", tag="b4")
            nc.sync.dma_start(b4_t[:], b4[:, :])
            idn_t = wpool.tile([P, P], F32, name="idn", tag="idn")
            nc.sync.dma_start(idn_t[:], idn[:, :])
            # f32r copies of W1 chunks (DVE copy is a valid f32r producer)
            w1r_t = []
            for i, kc in enumerate([128, 128, 128, 8]):
                wr = wpool.tile([kc, 270], MMDT, name=f"w1r_{i}",
                                tag=f"w1r_{i}")
                nc.vector.tensor_copy(wr[:], w1_t[i][:])
                w1r_t.append(wr)

            for ib in [i for _ in range(reps) for i in range(nb)]:
                # ---------------- load batch of T tiles
                X = xpool.tile([P, T * D], F32, name="x", tag="x")
                nc.sync.dma_start(
                    X[:].rearrange("p (t d) -> p t d", t=T),
                    t_in[P * T * ib:P * T * (ib + 1), :]
                    .rearrange("(t p) d -> p t d", p=P))

                F = fpool.tile([P, T * NF], F32, name="feat", tag="feat")
                Fv = F.rearrange("p (t f) -> p t f", t=T)

                def fsl(name):
                    return Fv[:, :, BOFF[name]:BOFF[name] + H]

                # mean_2 via contiguous windowed reduce on X
                X4 = X.rearrange("p (t r c) -> p t r c", r=H, c=H)
                nc.vector.tensor_reduce(fsl("mean_2"), X4, axis=AXX, op=ADD)

                Xi4 = X[:].bitcast(I32).rearrange("p (t r c) -> p t r c",
                                                  r=H, c=H)

                # double-width plane tiles + scratch pool (shared by axes)
                ptiles = [ppool.tile([P, 2 * FP], F32, name=f"pl_{k}",
                                     tag=f"pl_{k}") for k in range(14)]
                stiles = [ppool.tile([P, 2 * FP], F32, name=f"sc_{k}",
                                     tag=f"sc_{k}") for k in range(NSCR)]

                def sort_axis(axis):
                    """Plane build, plane-sum (for mean), leveled split sort,
                    pairing. Returns (minkey, medkey, maxkey, sum) APs."""
                    planes = [t[:] for t in ptiles]
                    scr = [t[:] for t in stiles]
                    for k in range(14):
                        for hb, e in ((0, k), (1, k + 14)):
                            src = (Xi4[:, :, e, :] if axis == 1
                                   else Xi4[:, :, :, e])
                            nc.vector.tensor_scalar(
                                planes[k][:, hb * FP:(hb + 1) * FP]
                                .bitcast(I32).rearrange("p (t c) -> p t c",
                                                        t=T),
                                src, -32, e,
                                op0=AluOpType.bitwise_and,
                                op1=AluOpType.bitwise_or)
                    # plane sum tree (mean from keys) into scr[0]
                    if axis == 1:
                        for k in range(7):
                            nc.vector.tensor_tensor(scr[k], planes[2 * k],
                                                    planes[2 * k + 1], op=ADD)
                        for (a, b) in ((0, 4), (1, 5), (2, 6), (0, 2),
                                       (1, 3), (0, 1)):
                            nc.vector.tensor_tensor(scr[a], scr[a], scr[b],
                                                    op=ADD)
                        nc.vector.tensor_tensor(
                            fsl("mean_1"), scr[0][:, 0:FP]
                            .rearrange("p (t c) -> p t c", t=T),
                            scr[0][:, FP:2 * FP]
                            .rearrange("p (t c) -> p t c", t=T), op=ADD)
                    # leveled split sort; all mins of a level, then all maxes
                    for lvl in NET14_LEVELS:
                        for ci, (i, j) in enumerate(lvl):
                            nc.vector.tensor_tensor(scr[ci], planes[i],
                                                    planes[j], op=MIN)
                        for ci, (i, j) in enumerate(lvl):
                            nc.vector.tensor_tensor(planes[j], planes[i],
                                                    planes[j], op=MAX)
                        for ci, (i, j) in enumerate(lvl):
                            scr[ci], planes[i] = planes[i], scr[ci]

                    def Ah(p):
                        return p[:, 0:FP]

                    def Bh(p):
                        return p[:, FP:2 * FP]

                    # min/max of union into scr[0] halves (before pairing
                    # clobbers the A halves in place)
                    w0 = Ah(scr[0])
                    w1_ = Bh(scr[0])
                    nc.vector.tensor_tensor(w0, Ah(planes[0]), Bh(planes[0]),
                                            op=MIN)
                    nc.vector.tensor_tensor(w1_, Ah(planes[13]),
                                            Bh(planes[13]), op=MAX)
                    # pairing mins in place on A halves (independent ops)
                    for i in range(14):
                        nc.vector.tensor_tensor(Ah(planes[i]), Ah(planes[i]),
                                                Bh(planes[13 - i]), op=MIN)
                    # tree max over pairing mins -> A[0] is the median key
                    n = 14
                    while n > 1:
                        h2 = n // 2
                        for i in range(h2):
                            nc.vector.tensor_tensor(
                                Ah(planes[i]), Ah(planes[i]),
                                Ah(planes[n - 1 - i]), op=MAX)
                        n = (n + 1) // 2
                    return w0, Ah(planes[0]), w1_

                idx_slots = []

                def extract(keyap, vname, iname):
                    k3 = keyap.rearrange("p (t c) -> p t c", t=T)
                    nc.scalar.copy(fsl(vname), k3)
                    nc.vector.tensor_scalar(
                        fsl(iname).bitcast(I32), k3.bitcast(I32),
                        31, IDX_BASE,
                        op0=AluOpType.bitwise_and, op1=AluOpType.bitwise_or)
                    idx_slots.append(iname)

                min1, med1, max1 = sort_axis(1)
                extract(min1, "min_v1", "min_i1")
                extract(max1, "max_v1", "max_i1")
                extract(med1, "med_v1", "med_i1")
                min2, med2, max2 = sort_axis(2)
                extract(min2, "min_v2", "min_i2")
                extract(max2, "max_v2", "max_i2")
                extract(med2, "med_v2", "med_i2")
                # idx features arrive as 2^23+idx; rebase to 0..27 on device
                # (folding 2^23 into b1 causes catastrophic cancellation)
                for iname in idx_slots:
                    nc.vector.tensor_scalar(
                        fsl(iname), fsl(iname), -8388608.0, None,
                        op0=AluOpType.add)

                if debug_features:
                    for til in range(T):
                        row0 = P * (T * ib + til)
                        nc.sync.dma_start(y_out[row0:row0 + P, :],
                                          Fv[:, til, :])
                    continue

                # ---------------- MLP on groups of 4 tiles
                ex_tiles = []
                for g in range(T // 4):
                    fT = []
                    for ci, (k0, kc) in enumerate([(0, 128), (128, 128),
                                                   (256, 128), (384, 8)]):
                        st2 = gpool.tile([P, 512], MMDT, name=f"fts_{g}_{ci}",
                                         tag=f"fts_{g % 2}_{ci}")
                        for tt in range(4):
                            til = g * 4 + tt
                            pt = psA.tile([P, P], F32, name=f"ftp_{ci}_{tt}",
                                          tag="ftp")
                            nc.tensor.transpose(pt[0:kc, :],
                                                Fv[:, til, k0:k0 + kc],
                                                idn_t[:])
                            nc.vector.tensor_copy(
                                st2[0:kc, 128 * tt:128 * (tt + 1)],
                                pt[0:kc, :])
                        fT.append(st2)

                    a1 = []
                    for mi, (m0, mc) in enumerate([(0, 128), (128, 128),
                                                   (256, 14)]):
                        ps = psB.tile([mc, 512], F32, name=f"l1_{m0}",
                                      tag=f"l1_{m0}")
                        for ci, (k0, kc) in enumerate([(0, 128), (128, 128),
                                                       (256, 128), (384, 8)]):
                            nc.tensor.matmul(
                                ps[:],
                                w1r_t[ci][0:kc, m0:m0 + mc],
                                fT[ci][0:kc, :],
                                start=(ci == 0), stop=(ci == 3))
                        sb = gpool.tile([mc, 512], F32, name=f"a1_{g}_{m0}",
                                        tag=f"a1_{g % 2}_{m0}")
                        nc.scalar.activation(sb[:], ps[:],
                                             mybir.ActivationFunctionType.Relu,
                                             bias=b1_t[mi][0:mc, :], scale=1.0)
                        a1.append(sb)

                    ps2 = psC.tile([128, 512], F32, name="l2",
                                   tag="mlps")[0:90, :]
                    for ci, kc in enumerate([128, 128, 14]):
                        nc.tensor.matmul(ps2[:], w2_t[ci][0:kc, :],
                                         a1[ci][0:kc, :],
                                         start=(ci == 0), stop=(ci == 2))
                    a2t = gpool.tile([90, 512], F32, name=f"a2_{g}",
                                     tag=f"a2_{g % 2}")
                    nc.scalar.activation(a2t[:], ps2[:],
                                         mybir.ActivationFunctionType.Relu,
                                         bias=b2_t[:], scale=1.0)

                    ps3 = psC.tile([128, 512], F32, name="l3",
                                   tag="mlps")[0:30, :]
                    nc.tensor.matmul(ps3[:], w3_t[:], a2t[:],
                                     start=True, stop=True)
                    a3t = gpool.tile([30, 512], F32, name=f"a3_{g}",
                                     tag=f"a3_{g % 2}")
                    nc.scalar.activation(a3t[:], ps3[:],
                                         mybir.ActivationFunctionType.Relu,
                                         bias=b3_t[:], scale=1.0)

                    ps4 = psC.tile([128, 512], F32, name="l4",
                                   tag="mlps")[0:10, :]
                    nc.tensor.matmul(ps4[:], w4_t[:], a3t[:],
                                     start=True, stop=True)
                    ex = gpool.tile([10, 512], F32, name=f"expt_{g}",
                                    tag=f"expt_{g % 2}")
                    nc.scalar.activation(ex[:], ps4[:],
                                         mybir.ActivationFunctionType.Exp,
                                         bias=b4_t[:], scale=1.0)
                    ex_tiles.append(ex)

                # batched softmax tail (wide op groups avoid sem stalls)
                psts, sumvs, rcps, yts = [], [], [], []
                for til in range(T):
                    ex = ex_tiles[til // 4]
                    tt = til % 4
                    pst = psA.tile([P, 16], F32, name=f"smT_{til}", tag="smT")
                    nc.tensor.transpose(pst[:, 0:10],
                                        ex[:, 128 * tt:128 * (tt + 1)],
                                        idn_t[0:10, 0:10])
                    psts.append(pst)
                for til in range(T):
                    sumv = mpool.tile([P, 1], F32, name=f"sumv_{til}",
                                      tag=f"sumv_{til}")
                    nc.vector.tensor_reduce(sumv[:], psts[til][:, 0:10],
                                            axis=AXX, op=ADD)
                    sumvs.append(sumv)
                for til in range(T):
                    rcp = mpool.tile([P, 1], F32, name=f"rcp_{til}",
                                     tag=f"rcp_{til}")
                    nc.vector.reciprocal(rcp[:], sumvs[til][:])
                    rcps.append(rcp)
                for til in range(T):
                    yt = mpool.tile([P, 16], F32, name=f"yt_{til}",
                                    tag=f"yt_{til}")
                    nc.vector.tensor_scalar_mul(yt[:, 0:10],
                                                psts[til][:, 0:10],
                                                rcps[til][:])
                    nc.vector.memzero(yt[:, 10:16])
                    yts.append(yt)
                for til in range(T):
                    row0 = P * (T * ib + til)
                    nc.sync.dma_start(y_out[row0:row0 + P, :], yts[til][:])

    _split_excess_waits(nc)
    return nc


# ------------------------------------------------------------- numpy driver
def _prep_weights(W1, b1, W2, b2, W3, b3, W4, b4):
    """Fold per-feature affine corrections into W1/b1; transpose for PE."""
    f64 = np.float64
    W1_eff = W1.astype(f64).copy()
    b1_eff = b1.astype(f64).copy()
    for bi, name in enumerate(BLOCKS):
        cols = slice(bi * H, (bi + 1) * H)
        if name in ("mean_1", "mean_2"):
            W1_eff[:, cols] /= H
    return {
        "w1": np.ascontiguousarray(W1_eff.T.astype(np.float32)),
        "b1": b1_eff.astype(np.float32).reshape(-1, 1),
        "w2": np.ascontiguousarray(W2.T.astype(np.float32)),
        "b2": b2.reshape(-1, 1).astype(np.float32),
        "w3": np.ascontiguousarray(W3.T.astype(np.float32)),
        "b3": b3.reshape(-1, 1).astype(np.float32),
        "w4": np.ascontiguousarray(W4.T.astype(np.float32)),
        "b4": b4.reshape(-1, 1).astype(np.float32),
        "idn": np.eye(P, dtype=np.float32),
    }


_NC_CACHE = {}


def _get_nc(n_tiles, **kw):
    key = (n_tiles, tuple(sorted(kw.items())))
    if key not in _NC_CACHE:
        _NC_CACHE[key] = build_nc(n_tiles, **kw)
    return _NC_CACHE[key]


def run(t, weights, n_tiles=N_TILES, trace=False, **kw):
    nc = _get_nc(n_tiles, **kw)
    rows = P * n_tiles
    in_maps = []
    for c in range(N_CORES):
        m = {"t": np.ascontiguousarray(t[c * B_CORE:c * B_CORE + rows])}
        m.update(weights)
        in_maps.append(m)
    res = run_bass_kernel_spmd(nc, in_maps, core_ids=list(range(N_CORES)),
                               trace=trace)
    outs = [r["y"] for r in res.results]
    return outs, res


def kernel(t, W1, b1, W2, b2, W3, b3, W4, b4):
    weights = _prep_weights(W1, b1, W2, b2, W3, b3, W4, b4)
    outs, _ = run(t, weights)
    y = np.concatenate([o[:, 0:10] for o in outs], axis=0)
    return np.ascontiguousarray(y.astype(np.float32))



# revision 9
# speedup vs baseline: 1.0129x; 1.0129x over previous
"""Trainium2 Bass kernel v3 for nn_CNNModel_42064909697048.

Strategy: f32-ordered sort keys with the group-local element index embedded
in the low 5 mantissa bits (value & ~31 | idx). One split sorting network per
axis (two Batcher-14 sorts + bitonic pairing) yields min/argmin, max/argmax,
median/argmedian simultaneously; means via windowed reduces; 4-layer MLP +
softmax on PE/ACT.

v4: all comparator work on DVE (the only engine with tensor_tensor on
TRN2); PSUM->SBUF copies, index rebases, value extracts and memzeros moved
to ACT; F double-buffered so batch ib+1's sort overlaps batch ib's MLP.
Data parallel over 8 NeuronCores; self-contained.
"""

import numpy as np

import concourse.bass as bass
import concourse.mybir as mybir
import concourse.tile as tile_mod
from concourse.tile import TileContext
from concourse.bass_utils import run_bass_kernel_spmd
from concourse.alu_op_type import AluOpType

# ---------------------------------------------------------------- constants
B_TOTAL = 131072
N_CORES = 8
B_CORE = B_TOTAL // N_CORES          # 16384
H = 28
D = 784
P = 128
N_TILES = B_CORE // P                # 128
T = 16                               # tiles per batch
FP = H * T                           # 224 slots per plane half
NF = 392
F32 = mybir.dt.float32
I32 = mybir.dt.int32
AXX = mybir.AxisListType.X
MIN = AluOpType.min
MAX = AluOpType.max
ADD = AluOpType.add
IDX_BASE = 0x4B000000                # f32 bits of 8388608.0; |idx -> 2^23+idx

BLOCKS = ["min_v1", "min_i1", "min_v2", "min_i2",
          "max_v1", "max_i1", "max_v2", "max_i2",
          "mean_1", "mean_2",
          "med_v1", "med_i1", "med_v2", "med_i2"]
BOFF = {k: i * H for i, k in enumerate(BLOCKS)}


def _batcher(n):
    m = 1
    while m < n:
        m *= 2
    net = []

    def merge(lo, cnt, r):
        step = r * 2
        if step < cnt:
            merge(lo, cnt, step)
            merge(lo + r, cnt, step)
            for i in range(lo + r, lo + cnt - r, step):
                net.append((i, i + r))
        else:
            net.append((lo, lo + r))

    def sort(lo, cnt):
        if cnt > 1:
            h2 = cnt // 2
            sort(lo, h2)
            sort(lo + h2, h2)
            merge(lo, cnt, 1)

    sort(0, m)
    return [(a, b) for (a, b) in net if b < n]


NET14 = _batcher(14)                 # 53 compare-exchanges


def _levels(net, n=14):
    """Group comparators into dependency levels for stall-free emission."""
    lvl_of = [0] * n
    levels = []
    for (i, j) in net:
        L = max(lvl_of[i], lvl_of[j])
        if L == len(levels):
            levels.append([])
        levels[L].append((i, j))
        lvl_of[i] = lvl_of[j] = L + 1
    return levels


NET14_LEVELS = _levels(NET14)
NSCR = 3                             # scratch tiles (rotated)

# ------------------------------------------------- tile tail-drain workaround
def _patched_drain_and_barrier(self, tick_clock, wait_clock):
    drain_inst = self.nc.sync.drain()
    wait_clock.add_sem_waits(
        drain_inst.ins, tile_mod.ScopedClock({None: tick_clock.global_clock})
    )
    si = drain_inst.ins.sync_info
    waits = list(si.on_wait or [])
    if len(waits) > 1:
        si.on_wait = waits[:1]
        for w in waits[1:]:
            d2 = self.nc.sync.drain()
            si2 = d2.ins.sync_info
            if si2 is None:
                d2.ins.sync_info = mybir.SyncInfo(on_wait=[w], on_update=[])
            else:
                si2.on_wait = [w]
    self.nc.all_engine_barrier()
    assert self.sems is not None
    popped = self.nc._tile_sem_poison_stack.pop()
    assert popped is self._sem_poison
    self.nc.clear_and_free_semaphores(list(self.sems.allocated().values()))
    self.nc.all_engine_barrier()


tile_mod.TileContext._drain_and_barrier = _patched_drain_and_barrier

MAX_WAITS = 1


def _split_excess_waits(nc):
    """Walrus in this container rejects instructions with >MAX_WAITS sem
    waits; hoist the excess onto NoOp carriers inserted just before."""
    import bass_rust
    ctr = [0]
    for f in nc.m.functions:
        for blk in f.blocks:
            insts = list(blk.instructions)
            out = []
            changed = False
            for inst in insts:
                si = inst.sync_info
                waits = list(si.on_wait) if (si and si.on_wait) else []
                if len(waits) > MAX_WAITS:
                    changed = True
                    excess = waits[:-MAX_WAITS]
                    si.on_wait = waits[-MAX_WAITS:]
                    for k in range(0, len(excess), MAX_WAITS):
                        nop = bass_rust.InstNoOp(
                            name=f"WSPLIT-{ctr[0]}", ins=[], outs=[])
                        ctr[0] += 1
                        nop.engine = inst.engine
                        nop.sync_info = mybir.SyncInfo(
                            on_wait=excess[k:k + MAX_WAITS], on_update=[])
                        out.append(nop)
                out.append(inst)
            if changed:
                blk.instructions = out


# ------------------------------------------------------------- bass program
def build_nc(n_tiles: int = N_TILES, mm_dtype: str = "f32r",
             debug_features: bool = False, reps: int = 1):
    nb = n_tiles // T
    nc = bass.Bass()
    t_in = nc.dram_tensor("t", [P * n_tiles, D], F32, kind="ExternalInput")
    w1 = nc.dram_tensor("w1", [NF, 270], F32, kind="ExternalInput")
    b1 = nc.dram_tensor("b1", [270, 1], F32, kind="ExternalInput")
    w2 = nc.dram_tensor("w2", [270, 90], F32, kind="ExternalInput")
    b2 = nc.dram_tensor("b2", [90, 1], F32, kind="ExternalInput")
    w3 = nc.dram_tensor("w3", [90, 30], F32, kind="ExternalInput")
    b3 = nc.dram_tensor("b3", [30, 1], F32, kind="ExternalInput")
    w4 = nc.dram_tensor("w4", [30, 10], F32, kind="ExternalInput")
    b4 = nc.dram_tensor("b4", [10, 1], F32, kind="ExternalInput")
    idn = nc.dram_tensor("idn", [P, P], F32, kind="ExternalInput")
    if debug_features:
        y_out = nc.dram_tensor("y", [P * n_tiles, NF], F32,
                               kind="ExternalOutput")
    else:
        y_out = nc.dram_tensor("y", [P * n_tiles, 16], F32,
                               kind="ExternalOutput")

    MMDT = {"f32": F32, "f32r": mybir.dt.float32r}[mm_dtype]

    with TileContext(nc) as tc:
        with (
            tc.tile_pool(name="wpool", bufs=1) as wpool,
            tc.tile_pool(name="xpool", bufs=1) as xpool,
            tc.tile_pool(name="ppool", bufs=1) as ppool,
            tc.tile_pool(name="fpool", bufs=2) as fpool,
            tc.tile_pool(name="gpool", bufs=1) as gpool,
            tc.tile_pool(name="mpool", bufs=2) as mpool,
            tc.tile_pool(name="psA", bufs=2, space="PSUM") as psA,
            tc.tile_pool(name="psB", bufs=1, space="PSUM") as psB,
            tc.tile_pool(name="psC", bufs=1, space="PSUM") as psC,
        ):
            # ---- static weights into SBUF
            w1_t = [wpool.tile([128, 270], F32, name=f"w1_{i}", tag=f"w1_{i}")
                    for i in range(3)]
            w1_t.append(wpool.tile([8, 270], F32, name="w1_3", tag="w1_3"))
            for i in range(3):
                nc.sync.dma_start(w1_t[i][:], w1[128 * i:128 * (i + 1), :])
            nc.sync.dma_start(w1_t[3][:], w1[384:392, :])
            w2_t = [wpool.tile([128, 90], F32, name="w2_0", tag="w2_0"),
                    wpool.tile([128, 90], F32, name="w2_1", tag="w2_1"),
                    wpool.tile([14, 90], F32, name="w2_2", tag="w2_2")]
            nc.sync.dma_start(w2_t[0][:], w2[0:128, :])
            nc.sync.dma_start(w2_t[1][:], w2[128:256, :])
            nc.sync.dma_start(w2_t[2][:], w2[256:270, :])
            w3_t = wpool.tile([90, 30], F32, name="w3", tag="w3")
            nc.sync.dma_start(w3_t[:], w3[:, :])
            w4_t = wpool.tile([30, 10], F32, name="w4", tag="w4")
            nc.sync.dma_start(w4_t[:], w4[:, :])
            b1_t = [wpool.tile([128, 1], F32, name="b1_0", tag="b1_0"),
                    wpool.tile([128, 1], F32, name="b1_1", tag="b1_1"),
                    wpool.tile([14, 1], F32, name="b1_2", tag="b1_2")]
            nc.sync.dma_start(b1_t[0][:], b1[0:128, :])
            nc.sync.dma_start(b1_t[1][:], b1[128:256, :])
            nc.sync.dma_start(b1_t[2][:], b1[256:270, :])
            b2_t = wpool.tile([90, 1], F32, name="b2", tag="b2")
            nc.sync.dma_start(b2_t[:], b2[:, :])
            b3_t = wpool.tile([30, 1], F32, name="b3", tag="b3")
            nc.sync.dma_start(b3_t[:], b3[:, :])
            b4_t = wpool.tile([10, 1], F32, name="b4", tag="b4")
            nc.sync.dma_start(b4_t[:], b4[:, :])
            idn_t = wpool.tile([P, P], F32, name="idn", tag="idn")
            nc.sync.dma_start(idn_t[:], idn[:, :])
            nidx_t = wpool.tile([P, 1], F32, name="nidx", tag="nidx")
            nc.vector.memset(nidx_t[:], -8388608.0)
            # f32r copies of W1 chunks (DVE copy is a valid f32r producer)
            w1r_t = []
            for i, kc in enumerate([128, 128, 128, 8]):
                wr = wpool.tile([kc, 270], MMDT, name=f"w1r_{i}",
                                tag=f"w1r_{i}")
                nc.vector.tensor_copy(wr[:], w1_t[i][:])
                w1r_t.append(wr)

            # double-width plane tiles + scratch (shared by both axes)
            pt1 = [ppool.tile([P, 2 * FP], F32, name=f"pl_{k}",
                              tag=f"pl_{k}") for k in range(14)]
            st1 = [ppool.tile([P, 2 * FP], F32, name=f"sc_{k}",
                              tag=f"sc_{k}") for k in range(NSCR)]

            for ib in [i for _ in range(reps) for i in range(nb)]:
                # ---------------- load batch of T tiles
                X = xpool.tile([P, T * D], F32, name="x", tag="x")
                nc.sync.dma_start(
                    X[:].rearrange("p (t d) -> p t d", t=T),
                    t_in[P * T * ib:P * T * (ib + 1), :]
                    .rearrange("(t p) d -> p t d", p=P))

                F = fpool.tile([P, T * NF], F32, name="feat", tag="feat")
                Fv = F.rearrange("p (t f) -> p t f", t=T)

                def fsl(name):
                    return Fv[:, :, BOFF[name]:BOFF[name] + H]

                # mean_2 via contiguous windowed reduce on X (DVE)
                X4 = X.rearrange("p (t r c) -> p t r c", r=H, c=H)
                nc.vector.tensor_reduce(fsl("mean_2"), X4, axis=AXX, op=ADD)

                Xi4 = X[:].bitcast(I32).rearrange("p (t r c) -> p t r c",
                                                  r=H, c=H)

                def sort_axis(axis, ptiles, stiles):
                    """Plane build, plane-sum (mean, axis1), leveled split
                    sort, pairing — all on DVE. Returns
                    (minkey, medkey, maxkey) APs."""
                    planes = [t[:] for t in ptiles]
                    scr = [t[:] for t in stiles]
                    for k in range(14):
                        for hb, e in ((0, k), (1, k + 14)):
                            src = (Xi4[:, :, e, :] if axis == 1
                                   else Xi4[:, :, :, e])
                            nc.vector.tensor_scalar(
                                planes[k][:, hb * FP:(hb + 1) * FP]
                                .bitcast(I32).rearrange("p (t c) -> p t c",
                                                        t=T),
                                src, -32, e,
                                op0=AluOpType.bitwise_and,
                                op1=AluOpType.bitwise_or)
                    # plane sum tree (mean from keys), axis 1 only
                    if axis == 1:
                        tt = nc.vector.tensor_tensor
                        tt(scr[0], planes[0], planes[1], op=ADD)
                        tt(scr[1], planes[2], planes[3], op=ADD)
                        tt(scr[2], planes[4], planes[5], op=ADD)
                        tt(scr[0], scr[0], scr[1], op=ADD)
                        tt(scr[1], planes[6], planes[7], op=ADD)
                        tt(scr[0], scr[0], scr[2], op=ADD)
                        tt(scr[2], planes[8], planes[9], op=ADD)
                        tt(scr[0], scr[0], scr[1], op=ADD)
                        tt(scr[1], planes[10], planes[11], op=ADD)
                        tt(scr[0], scr[0], scr[2], op=ADD)
                        tt(scr[2], planes[12], planes[13], op=ADD)
                        tt(scr[0], scr[0], scr[1], op=ADD)
                        tt(scr[0], scr[0], scr[2], op=ADD)
                        tt(fsl("mean_1"), scr[0][:, 0:FP]
                           .rearrange("p (t c) -> p t c", t=T),
                           scr[0][:, FP:2 * FP]
                           .rearrange("p (t c) -> p t c", t=T), op=ADD)
                    # leveled split sort; all mins of a level, then all maxes
                    free = list(scr)
                    for lvl in NET14_LEVELS:
                        for (i, j) in lvl:
                            s = free.pop(0)
                            nc.vector.tensor_tensor(s, planes[i], planes[j],
                                                    op=MIN)
                            nc.vector.tensor_tensor(planes[j], planes[i],
                                                    planes[j], op=MAX)
                            free.append(planes[i])
                            planes[i] = s
                    scr = free

                    def Ah(p):
                        return p[:, 0:FP]

                    def Bh(p):
                        return p[:, FP:2 * FP]

                    # min/max of union into scr[0] halves (before pairing
                    # clobbers the A halves in place)
                    w0 = Ah(scr[0])
                    w1_ = Bh(scr[0])
                    nc.vector.tensor_tensor(w0, Ah(planes[0]), Bh(planes[0]),
                                            op=MIN)
                    nc.vector.tensor_tensor(w1_, Ah(planes[13]),
                                            Bh(planes[13]), op=MAX)
                    # pairing mins in place on A halves (independent ops)
                    for i in range(14):
                        nc.vector.tensor_tensor(Ah(planes[i]), Ah(planes[i]),
                                                Bh(planes[13 - i]), op=MIN)
                    # tree max over pairing mins -> A[0] is the median key
                    n = 14
                    while n > 1:
                        h2 = n // 2
                        for i in range(h2):
                            nc.vector.tensor_tensor(
                                Ah(planes[i]), Ah(planes[i]),
                                Ah(planes[n - 1 - i]), op=MAX)
                        n = (n + 1) // 2
                    return w0, Ah(planes[0]), w1_

                idx_slots = []

                def extract(keyap, vname, iname):
                    k3 = keyap.rearrange("p (t c) -> p t c", t=T)
                    nc.scalar.copy(fsl(vname), k3)
                    nc.vector.tensor_scalar(
                        fsl(iname).bitcast(I32), k3.bitcast(I32),
                        31, IDX_BASE,
                        op0=AluOpType.bitwise_and, op1=AluOpType.bitwise_or)
                    idx_slots.append(iname)

                min1, med1, max1 = sort_axis(1, pt1, st1)
                extract(min1, "min_v1", "min_i1")
                extract(max1, "max_v1", "max_i1")
                extract(med1, "med_v1", "med_i1")
                min2, med2, max2 = sort_axis(2, pt1, st1)
                extract(min2, "min_v2", "min_i2")
                extract(max2, "max_v2", "max_i2")
                extract(med2, "med_v2", "med_i2")
                # idx features arrive as 2^23+idx; rebase to 0..27 on ACT
                # (folding 2^23 into b1 causes catastrophic cancellation)
                for iname in idx_slots:
                    nc.scalar.activation(
                        fsl(iname), fsl(iname),
                        mybir.ActivationFunctionType.Identity,
                        bias=nidx_t[:], scale=1.0)

                if debug_features:
                    for til in range(T):
                        row0 = P * (T * ib + til)
                        nc.sync.dma_start(y_out[row0:row0 + P, :],
                                          Fv[:, til, :])
                    continue

                # ---------------- MLP on groups of 4 tiles
                ex_tiles = []
                for g in range(T // 4):
                    fT = []
                    for ci, (k0, kc) in enumerate([(0, 128), (128, 128),
                                                   (256, 128), (384, 8)]):
                        st2 = gpool.tile([P, 512], MMDT, name=f"fts_{g}_{ci}",
                                         tag=f"fts_{ci}")
                        for tt in range(4):
                            til = g * 4 + tt
                            pt = psA.tile([P, P], F32, name=f"ftp_{ci}_{tt}",
                                          tag="ftp")
                            nc.tensor.transpose(pt[0:kc, :],
                                                Fv[:, til, k0:k0 + kc],
                                                idn_t[:])
                            nc.scalar.copy(
                                st2[0:kc, 128 * tt:128 * (tt + 1)],
                                pt[0:kc, :])
                        fT.append(st2)

                    a1 = []
                    for mi, (m0, mc) in enumerate([(0, 128), (128, 128),
                                                   (256, 14)]):
                        ps = psB.tile([mc, 512], F32, name=f"l1_{m0}",
                                      tag=f"l1_{m0}")
                        for ci, (k0, kc) in enumerate([(0, 128), (128, 128),
                                                       (256, 128), (384, 8)]):
                            nc.tensor.matmul(
                                ps[:],
                                w1r_t[ci][0:kc, m0:m0 + mc],
                                fT[ci][0:kc, :],
                                start=(ci == 0), stop=(ci == 3))
                        sb = gpool.tile([mc, 512], F32, name=f"a1_{g}_{m0}",
                                        tag=f"a1_{m0}")
                        nc.scalar.activation(sb[:], ps[:],
                                             mybir.ActivationFunctionType.Relu,
                                             bias=b1_t[mi][0:mc, :], scale=1.0)
                        a1.append(sb)

                    ps2 = psC.tile([128, 512], F32, name="l2",
                                   tag="mlps")[0:90, :]
                    for ci, kc in enumerate([128, 128, 14]):
                        nc.tensor.matmul(ps2[:], w2_t[ci][0:kc, :],
                                         a1[ci][0:kc, :],
                                         start=(ci == 0), stop=(ci == 2))
                    a2t = gpool.tile([90, 512], F32, name=f"a2_{g}",
                                     tag="a2")
                    nc.scalar.activation(a2t[:], ps2[:],
                                         mybir.ActivationFunctionType.Relu,
                                         bias=b2_t[:], scale=1.0)

                    ps3 = psC.tile([128, 512], F32, name="l3",
                                   tag="mlps")[0:30, :]
                    nc.tensor.matmul(ps3[:], w3_t[:], a2t[:],
                                     start=True, stop=True)
                    a3t = gpool.tile([30, 512], F32, name=f"a3_{g}",
                                     tag="a3")
                    nc.scalar.activation(a3t[:], ps3[:],
                                         mybir.ActivationFunctionType.Relu,
                                         bias=b3_t[:], scale=1.0)

                    ps4 = psC.tile([128, 512], F32, name="l4",
                                   tag="mlps")[0:10, :]
                    nc.tensor.matmul(ps4[:], w4_t[:], a3t[:],
                                     start=True, stop=True)
                    ex = gpool.tile([10, 512], F32, name=f"expt_{g}",
                                    tag="expt")
                    nc.scalar.activation(ex[:], ps4[:],
                                         mybir.ActivationFunctionType.Exp,
                                         bias=b4_t[:], scale=1.0)
                    ex_tiles.append(ex)

                # batched softmax tail (wide op groups avoid sem stalls)
                psts, sumvs, rcps, yts = [], [], [], []
                for til in range(T):
                    ex = ex_tiles[til // 4]
                    tt = til % 4
                    pst = psA.tile([P, 16], F32, name=f"smT_{til}", tag="smT")
                    nc.tensor.transpose(pst[:, 0:10],
                                        ex[:, 128 * tt:128 * (tt + 1)],
                                        idn_t[0:10, 0:10])
                    psts.append(pst)
                for til in range(T):
                    sumv = mpool.tile([P, 1], F32, name=f"sumv_{til}",
                                      tag=f"sumv_{til}")
                    nc.vector.tensor_reduce(sumv[:], psts[til][:, 0:10],
                                            axis=AXX, op=ADD)
                    sumvs.append(sumv)
                for til in range(T):
                    rcp = mpool.tile([P, 1], F32, name=f"rcp_{til}",
                                     tag=f"rcp_{til}")
                    nc.vector.reciprocal(rcp[:], sumvs[til][:])
                    rcps.append(rcp)
                for til in range(T):
                    yt = mpool.tile([P, 16], F32, name=f"yt_{til}",
                                    tag=f"yt_{til}")
                    nc.vector.tensor_scalar_mul(yt[:, 0:10],
                                                psts[til][:, 0:10],
                                                rcps[til][:])
                    nc.scalar.memzero(yt[:, 10:16])
                    yts.append(yt)
                for til in range(T):
                    row0 = P * (T * ib + til)
                    nc.sync.dma_start(y_out[row0:row0 + P, :], yts[til][:])

    _split_excess_waits(nc)
    return nc


# ------------------------------------------------------------- numpy driver
def _prep_weights(W1, b1, W2, b2, W3, b3, W4, b4):
    """Fold per-feature affine corrections into W1/b1; transpose for PE."""
    f64 = np.float64
    W1_eff = W1.astype(f64).copy()
    b1_eff = b1.astype(f64).copy()
    for bi, name in enumerate(BLOCKS):
        cols = slice(bi * H, (bi + 1) * H)
        if name in ("mean_1", "mean_2"):
            W1_eff[:, cols] /= H
    return {
        "w1": np.ascontiguousarray(W1_eff.T.astype(np.float32)),
        "b1": b1_eff.astype(np.float32).reshape(-1, 1),
        "w2": np.ascontiguousarray(W2.T.astype(np.float32)),
        "b2": b2.reshape(-1, 1).astype(np.float32),
        "w3": np.ascontiguousarray(W3.T.astype(np.float32)),
        "b3": b3.reshape(-1, 1).astype(np.float32),
        "w4": np.ascontiguousarray(W4.T.astype(np.float32)),
        "b4": b4.reshape(-1, 1).astype(np.float32),
        "idn": np.eye(P, dtype=np.float32),
    }


_NC_CACHE = {}


def _get_nc(n_tiles, **kw):
    key = (n_tiles, tuple(sorted(kw.items())))
    if key not in _NC_CACHE:
        _NC_CACHE[key] = build_nc(n_tiles, **kw)
    return _NC_CACHE[key]


def run(t, weights, n_tiles=N_TILES, trace=False, **kw):
    nc = _get_nc(n_tiles, **kw)
    rows = P * n_tiles
    in_maps = []
    for c in range(N_CORES):
        m = {"t": np.ascontiguousarray(t[c * B_CORE:c * B_CORE + rows])}
        m.update(weights)
        in_maps.append(m)
    res = run_bass_kernel_spmd(nc, in_maps, core_ids=list(range(N_CORES)),
                               trace=trace)
    outs = [r["y"] for r in res.results]
    return outs, res


def kernel(t, W1, b1, W2, b2, W3, b3, W4, b4):
    weights = _prep_weights(W1, b1, W2, b2, W3, b3, W4, b4)
    outs, _ = run(t, weights)
    y = np.concatenate([o[:, 0:10] for o in outs], axis=0)
    return np.ascontiguousarray(y.astype(np.float32))


# revision 13
# speedup vs baseline: 1.2303x; 1.2147x over previous
"""Trainium2 Bass kernel v3 for nn_CNNModel_42064909697048.

Strategy: f32-ordered sort keys with the group-local element index embedded
in the low 5 mantissa bits (value & ~31 | idx). One split sorting network per
axis (two Batcher-14 sorts + bitonic pairing) yields min/argmin, max/argmax,
median/argmedian simultaneously; means via windowed reduces; 4-layer MLP +
softmax on PE/ACT.

v4: all comparator work on DVE (the only engine with tensor_tensor on
TRN2); PSUM->SBUF copies, index rebases, value extracts and memzeros moved
to ACT; F double-buffered so batch ib+1's sort overlaps batch ib's MLP.
Data parallel over 8 NeuronCores; self-contained.
"""

import numpy as np

import concourse.bass as bass
import concourse.mybir as mybir
import concourse.tile as tile_mod
from concourse.tile import TileContext
from concourse.bass_utils import run_bass_kernel_spmd
from concourse.alu_op_type import AluOpType

# ---------------------------------------------------------------- constants
B_TOTAL = 131072
N_CORES = 8
B_CORE = B_TOTAL // N_CORES          # 16384
H = 28
D = 784
P = 128
N_TILES = B_CORE // P                # 128
T = 16                               # tiles per batch
FP = H * T                           # 224 slots per plane half
NF = 392
F32 = mybir.dt.float32
I32 = mybir.dt.int32
AXX = mybir.AxisListType.X
MIN = AluOpType.min
MAX = AluOpType.max
ADD = AluOpType.add
IDX_BASE = 0x4B000000                # f32 bits of 8388608.0; |idx -> 2^23+idx

BLOCKS = ["min_v1", "min_i1", "min_v2", "min_i2",
          "max_v1", "max_i1", "max_v2", "max_i2",
          "mean_1", "mean_2",
          "med_v1", "med_i1", "med_v2", "med_i2"]
BOFF = {k: i * H for i, k in enumerate(BLOCKS)}


def _batcher(n):
    m = 1
    while m < n:
        m *= 2
    net = []

    def merge(lo, cnt, r):
        step = r * 2
        if step < cnt:
            merge(lo, cnt, step)
            merge(lo + r, cnt, step)
            for i in range(lo + r, lo + cnt - r, step):
                net.append((i, i + r))
        else:
            net.append((lo, lo + r))

    def sort(lo, cnt):
        if cnt > 1:
            h2 = cnt // 2
            sort(lo, h2)
            sort(lo + h2, h2)
            merge(lo, cnt, 1)

    sort(0, m)
    return [(a, b) for (a, b) in net if b < n]


NET14 = _batcher(14)                 # 53 compare-exchanges


def _levels(net, n=14):
    """Group comparators into dependency levels for stall-free emission."""
    lvl_of = [0] * n
    levels = []
    for (i, j) in net:
        L = max(lvl_of[i], lvl_of[j])
        if L == len(levels):
            levels.append([])
        levels[L].append((i, j))
        lvl_of[i] = lvl_of[j] = L + 1
    return levels


NET14_LEVELS = _levels(NET14)
NSCR = 2                             # scratch tiles (rotated)

# ------------------------------------------------- tile tail-drain workaround
def _patched_drain_and_barrier(self, tick_clock, wait_clock):
    drain_inst = self.nc.sync.drain()
    wait_clock.add_sem_waits(
        drain_inst.ins, tile_mod.ScopedClock({None: tick_clock.global_clock})
    )
    si = drain_inst.ins.sync_info
    waits = list(si.on_wait or [])
    if len(waits) > 1:
        si.on_wait = waits[:1]
        for w in waits[1:]:
            d2 = self.nc.sync.drain()
            si2 = d2.ins.sync_info
            if si2 is None:
                d2.ins.sync_info = mybir.SyncInfo(on_wait=[w], on_update=[])
            else:
                si2.on_wait = [w]
    self.nc.all_engine_barrier()
    assert self.sems is not None
    popped = self.nc._tile_sem_poison_stack.pop()
    assert popped is self._sem_poison
    self.nc.clear_and_free_semaphores(list(self.sems.allocated().values()))
    self.nc.all_engine_barrier()


tile_mod.TileContext._drain_and_barrier = _patched_drain_and_barrier

MAX_WAITS = 1


def _split_excess_waits(nc):
    """Walrus in this container rejects instructions with >MAX_WAITS sem
    waits; hoist the excess onto NoOp carriers inserted just before."""
    import bass_rust
    ctr = [0]
    for f in nc.m.functions:
        for blk in f.blocks:
            insts = list(blk.instructions)
            out = []
            changed = False
            for inst in insts:
                si = inst.sync_info
                waits = list(si.on_wait) if (si and si.on_wait) else []
                if len(waits) > MAX_WAITS:
                    changed = True
                    excess = waits[:-MAX_WAITS]
                    si.on_wait = waits[-MAX_WAITS:]
                    for k in range(0, len(excess), MAX_WAITS):
                        nop = bass_rust.InstNoOp(
                            name=f"WSPLIT-{ctr[0]}", ins=[], outs=[])
                        ctr[0] += 1
                        nop.engine = inst.engine
                        nop.sync_info = mybir.SyncInfo(
                            on_wait=excess[k:k + MAX_WAITS], on_update=[])
                        out.append(nop)
                out.append(inst)
            if changed:
                blk.instructions = out


# ------------------------------------------------------------- bass program
def build_nc(n_tiles: int = N_TILES, mm_dtype: str = "f32r",
             debug_features: bool = False, reps: int = 1):
    nb = n_tiles // T
    nc = bass.Bass()
    t_in = nc.dram_tensor("t", [P * n_tiles, D], F32, kind="ExternalInput")
    w1 = nc.dram_tensor("w1", [NF, 270], F32, kind="ExternalInput")
    b1 = nc.dram_tensor("b1", [270, 1], F32, kind="ExternalInput")
    w2 = nc.dram_tensor("w2", [270, 90], F32, kind="ExternalInput")
    b2 = nc.dram_tensor("b2", [90, 1], F32, kind="ExternalInput")
    w3 = nc.dram_tensor("w3", [90, 30], F32, kind="ExternalInput")
    b3 = nc.dram_tensor("b3", [30, 1], F32, kind="ExternalInput")
    w4 = nc.dram_tensor("w4", [30, 10], F32, kind="ExternalInput")
    b4 = nc.dram_tensor("b4", [10, 1], F32, kind="ExternalInput")
    idn = nc.dram_tensor("idn", [P, P], F32, kind="ExternalInput")
    if debug_features:
        y_out = nc.dram_tensor("y", [P * n_tiles, NF], F32,
                               kind="ExternalOutput")
    else:
        y_out = nc.dram_tensor("y", [P * n_tiles, 16], F32,
                               kind="ExternalOutput")

    MMDT = {"f32": F32, "f32r": mybir.dt.float32r}[mm_dtype]

    with TileContext(nc) as tc:
        with (
            tc.tile_pool(name="wpool", bufs=1) as wpool,
            tc.tile_pool(name="xpool", bufs=1) as xpool,
            tc.tile_pool(name="ppool", bufs=1) as ppool,
            tc.tile_pool(name="fpool", bufs=2) as fpool,
            tc.tile_pool(name="gpool", bufs=1) as gpool,
            tc.tile_pool(name="mpool", bufs=1) as mpool,
            tc.tile_pool(name="psA", bufs=2, space="PSUM") as psA,
            tc.tile_pool(name="psB", bufs=1, space="PSUM") as psB,
            tc.tile_pool(name="psC", bufs=1, space="PSUM") as psC,
        ):
            # ---- static weights into SBUF
            w1_t = [wpool.tile([128, 270], F32, name=f"w1_{i}", tag=f"w1_{i}")
                    for i in range(3)]
            w1_t.append(wpool.tile([8, 270], F32, name="w1_3", tag="w1_3"))
            for i in range(3):
                nc.sync.dma_start(w1_t[i][:], w1[128 * i:128 * (i + 1), :])
            nc.sync.dma_start(w1_t[3][:], w1[384:392, :])
            w2_t = [wpool.tile([128, 90], F32, name="w2_0", tag="w2_0"),
                    wpool.tile([128, 90], F32, name="w2_1", tag="w2_1"),
                    wpool.tile([14, 90], F32, name="w2_2", tag="w2_2")]
            nc.sync.dma_start(w2_t[0][:], w2[0:128, :])
            nc.sync.dma_start(w2_t[1][:], w2[128:256, :])
            nc.sync.dma_start(w2_t[2][:], w2[256:270, :])
            w3_t = wpool.tile([90, 30], F32, name="w3", tag="w3")
            nc.sync.dma_start(w3_t[:], w3[:, :])
            w4_t = wpool.tile([30, 10], F32, name="w4", tag="w4")
            nc.sync.dma_start(w4_t[:], w4[:, :])
            b1_t = [wpool.tile([128, 1], F32, name="b1_0", tag="b1_0"),
                    wpool.tile([128, 1], F32, name="b1_1", tag="b1_1"),
                    wpool.tile([14, 1], F32, name="b1_2", tag="b1_2")]
            nc.sync.dma_start(b1_t[0][:], b1[0:128, :])
            nc.sync.dma_start(b1_t[1][:], b1[128:256, :])
            nc.sync.dma_start(b1_t[2][:], b1[256:270, :])
            b2_t = wpool.tile([90, 1], F32, name="b2", tag="b2")
            nc.sync.dma_start(b2_t[:], b2[:, :])
            b3_t = wpool.tile([30, 1], F32, name="b3", tag="b3")
            nc.sync.dma_start(b3_t[:], b3[:, :])
            b4_t = wpool.tile([10, 1], F32, name="b4", tag="b4")
            nc.sync.dma_start(b4_t[:], b4[:, :])
            idn_t = wpool.tile([P, P], F32, name="idn", tag="idn")
            nc.sync.dma_start(idn_t[:], idn[:, :])
            nidx_t = wpool.tile([P, 1], F32, name="nidx", tag="nidx")
            nc.vector.memset(nidx_t[:], -8388608.0)
            # f32r copies of W1 chunks (DVE copy is a valid f32r producer)
            w1r_t = []
            for i, kc in enumerate([128, 128, 128, 8]):
                wr = wpool.tile([kc, 270], MMDT, name=f"w1r_{i}",
                                tag=f"w1r_{i}")
                nc.vector.tensor_copy(wr[:], w1_t[i][:])
                w1r_t.append(wr)

            # double-width plane tiles + scratch (shared by both axes)
            pt1 = [ppool.tile([P, 2 * FP], F32, name=f"pl_{k}",
                              tag=f"pl_{k}") for k in range(14)]
            st1 = [ppool.tile([P, 2 * FP], F32, name=f"sc_{k}",
                              tag=f"sc_{k}") for k in range(NSCR)]

            def softmax_tail(ib, pst):
                # DVE/ACT tail for batch ib, emitted one batch later so the
                # DVE stream never waits on batch ib's PE/ACT MLP chain.
                pv = pst[:].rearrange("p (t c) -> p t c", t=T)
                sumv = mpool.tile([P, 16], F32, name="sumv", tag="sumv")
                nc.vector.tensor_reduce(sumv[:], pv[:, :, 0:10],
                                        axis=AXX, op=ADD)
                rcp = mpool.tile([P, 16], F32, name="rcp", tag="rcp")
                nc.vector.reciprocal(rcp[:], sumv[:])
                yts = []
                for til in range(T):
                    yt = mpool.tile([P, 16], F32, name=f"yt_{til}",
                                    tag=f"yt_{til}")
                    nc.vector.tensor_scalar_mul(yt[:, 0:10],
                                                pv[:, til, 0:10],
                                                rcp[:, til:til + 1])
                    if ib < 2:
                        nc.scalar.memzero(yt[:, 10:16])
                    yts.append(yt)
                for til in range(T):
                    row0 = P * (T * ib + til)
                    nc.sync.dma_start(y_out[row0:row0 + P, :], yts[til][:])

            pending = None
            for ib in [i for _ in range(reps) for i in range(nb)]:
                # ---------------- load batch of T tiles
                X = xpool.tile([P, T * D], F32, name="x", tag="x")
                nc.sync.dma_start(
                    X[:].rearrange("p (t d) -> p t d", t=T),
                    t_in[P * T * ib:P * T * (ib + 1), :]
                    .rearrange("(t p) d -> p t d", p=P))

                F = fpool.tile([P, T * NF], F32, name="feat", tag="feat")
                Fv = F.rearrange("p (t f) -> p t f", t=T)

                def fsl(name):
                    return Fv[:, :, BOFF[name]:BOFF[name] + H]

                # mean_2 via contiguous windowed reduce on X (DVE)
                X4 = X.rearrange("p (t r c) -> p t r c", r=H, c=H)
                nc.vector.tensor_reduce(fsl("mean_2"), X4, axis=AXX, op=ADD)

                Xi4 = X[:].bitcast(I32).rearrange("p (t r c) -> p t r c",
                                                  r=H, c=H)

                def sort_axis(axis, ptiles, stiles):
                    """Plane build, plane-sum (mean, axis1), leveled split
                    sort, pairing — all on DVE. Returns
                    (minkey, medkey, maxkey) APs."""
                    planes = [t[:] for t in ptiles]
                    scr = [t[:] for t in stiles]
                    for k in range(14):
                        for hb, e in ((0, k), (1, k + 14)):
                            src = (Xi4[:, :, e, :] if axis == 1
                                   else Xi4[:, :, :, e])
                            nc.vector.tensor_scalar(
                                planes[k][:, hb * FP:(hb + 1) * FP]
                                .bitcast(I32).rearrange("p (t c) -> p t c",
                                                        t=T),
                                src, -32, e,
                                op0=AluOpType.bitwise_and,
                                op1=AluOpType.bitwise_or)
                    # plane sum tree (mean from keys), axis 1 only
                    if axis == 1:
                        tt = nc.vector.tensor_tensor
                        tt(scr[0], planes[0], planes[1], op=ADD)
                        for k in range(1, 7):
                            tt(scr[1], planes[2 * k], planes[2 * k + 1],
                               op=ADD)
                            tt(scr[0], scr[0], scr[1], op=ADD)
                        tt(fsl("mean_1"), scr[0][:, 0:FP]
                           .rearrange("p (t c) -> p t c", t=T),
                           scr[0][:, FP:2 * FP]
                           .rearrange("p (t c) -> p t c", t=T), op=ADD)
                    # leveled split sort; all mins of a level, then all maxes
                    free = list(scr)
                    for lvl in NET14_LEVELS:
                        for (i, j) in lvl:
                            s = free.pop(0)
                            nc.vector.tensor_tensor(s, planes[i], planes[j],
                                                    op=MIN)
                            nc.vector.tensor_tensor(planes[j], planes[i],
                                                    planes[j], op=MAX)
                            free.append(planes[i])
                            planes[i] = s
                    scr = free

                    def Ah(p):
                        return p[:, 0:FP]

                    def Bh(p):
                        return p[:, FP:2 * FP]

                    # min/max of union into scr[0] halves (before pairing
                    # clobbers the A halves in place)
                    w0 = Ah(scr[0])
                    w1_ = Bh(scr[0])
                    nc.vector.tensor_tensor(w0, Ah(planes[0]), Bh(planes[0]),
                                            op=MIN)
                    nc.vector.tensor_tensor(w1_, Ah(planes[13]),
                                            Bh(planes[13]), op=MAX)
                    # pairing mins in place on A halves (independent ops)
                    for i in range(14):
                        nc.vector.tensor_tensor(Ah(planes[i]), Ah(planes[i]),
                                                Bh(planes[13 - i]), op=MIN)
                    # tree max over pairing mins -> A[0] is the median key
                    n = 14
                    while n > 1:
                        h2 = n // 2
                        for i in range(h2):
                            nc.vector.tensor_tensor(
                                Ah(planes[i]), Ah(planes[i]),
                                Ah(planes[n - 1 - i]), op=MAX)
                        n = (n + 1) // 2
                    return w0, Ah(planes[0]), w1_

                idx_slots = []

                def extract(keyap, vname, iname):
                    k3 = keyap.rearrange("p (t c) -> p t c", t=T)
                    nc.scalar.copy(fsl(vname), k3)
                    nc.vector.tensor_scalar(
                        fsl(iname).bitcast(I32), k3.bitcast(I32),
                        31, IDX_BASE,
                        op0=AluOpType.bitwise_and, op1=AluOpType.bitwise_or)
                    idx_slots.append(iname)

                min1, med1, max1 = sort_axis(1, pt1, st1)
                extract(min1, "min_v1", "min_i1")
                extract(max1, "max_v1", "max_i1")
                extract(med1, "med_v1", "med_i1")
                min2, med2, max2 = sort_axis(2, pt1, st1)
                extract(min2, "min_v2", "min_i2")
                extract(max2, "max_v2", "max_i2")
                extract(med2, "med_v2", "med_i2")
                # idx features arrive as 2^23+idx; rebase to 0..27 on ACT
                # (folding 2^23 into b1 causes catastrophic cancellation)
                for iname in idx_slots:
                    nc.scalar.activation(
                        fsl(iname), fsl(iname),
                        mybir.ActivationFunctionType.Identity,
                        bias=nidx_t[:], scale=1.0)

                if pending is not None:
                    softmax_tail(*pending)
                    pending = None

                if debug_features:
                    for til in range(T):
                        row0 = P * (T * ib + til)
                        nc.sync.dma_start(y_out[row0:row0 + P, :],
                                          Fv[:, til, :])
                    continue

                # ---------------- MLP on groups of 4 tiles
                ex_tiles = []
                for g in range(T // 4):
                    fT = []
                    for ci, (k0, kc) in enumerate([(0, 128), (128, 128),
                                                   (256, 128), (384, 8)]):
                        st2 = gpool.tile([P, 512], MMDT, name=f"fts_{g}_{ci}",
                                         tag=f"fts_{g % 2}_{ci}")
                        for tt in range(4):
                            til = g * 4 + tt
                            pt = psA.tile([P, P], F32, name=f"ftp_{ci}_{tt}",
                                          tag="ftp")
                            nc.tensor.transpose(pt[0:kc, :],
                                                Fv[:, til, k0:k0 + kc],
                                                idn_t[:])
                            nc.scalar.copy(
                                st2[0:kc, 128 * tt:128 * (tt + 1)],
                                pt[0:kc, :])
                        fT.append(st2)

                    a1 = []
                    for mi, (m0, mc) in enumerate([(0, 128), (128, 128),
                                                   (256, 14)]):
                        ps = psB.tile([mc, 512], F32, name=f"l1_{m0}",
                                      tag=f"l1_{m0}")
                        for ci, (k0, kc) in enumerate([(0, 128), (128, 128),
                                                       (256, 128), (384, 8)]):
                            nc.tensor.matmul(
                                ps[:],
                                w1r_t[ci][0:kc, m0:m0 + mc],
                                fT[ci][0:kc, :],
                                start=(ci == 0), stop=(ci == 3))
                        sb = gpool.tile([mc, 512], F32, name=f"a1_{g}_{m0}",
                                        tag=f"a1_{g % 2}_{m0}")
                        nc.scalar.activation(sb[:], ps[:],
                                             mybir.ActivationFunctionType.Relu,
                                             bias=b1_t[mi][0:mc, :], scale=1.0)
                        a1.append(sb)

                    ps2 = psC.tile([128, 512], F32, name="l2",
                                   tag="mlps")[0:90, :]
                    for ci, kc in enumerate([128, 128, 14]):
                        nc.tensor.matmul(ps2[:], w2_t[ci][0:kc, :],
                                         a1[ci][0:kc, :],
                                         start=(ci == 0), stop=(ci == 2))
                    a2t = gpool.tile([90, 512], F32, name=f"a2_{g}",
                                     tag=f"a2_{g % 2}")
                    nc.scalar.activation(a2t[:], ps2[:],
                                         mybir.ActivationFunctionType.Relu,
                                         bias=b2_t[:], scale=1.0)

                    ps3 = psC.tile([128, 512], F32, name="l3",
                                   tag="mlps")[0:30, :]
                    nc.tensor.matmul(ps3[:], w3_t[:], a2t[:],
                                     start=True, stop=True)
                    a3t = gpool.tile([30, 512], F32, name=f"a3_{g}",
                                     tag=f"a3_{g % 2}")
                    nc.scalar.activation(a3t[:], ps3[:],
                                         mybir.ActivationFunctionType.Relu,
                                         bias=b3_t[:], scale=1.0)

                    ps4 = psC.tile([128, 512], F32, name="l4",
                                   tag="mlps")[0:10, :]
                    nc.tensor.matmul(ps4[:], w4_t[:], a3t[:],
                                     start=True, stop=True)
                    ex = gpool.tile([10, 512], F32, name=f"expt_{g}",
                                    tag=f"expt_{g % 2}")
                    nc.scalar.activation(ex[:], ps4[:],
                                         mybir.ActivationFunctionType.Exp,
                                         bias=b4_t[:], scale=1.0)
                    ex_tiles.append(ex)

                # softmax transposes (PE) into one packed PSUM tile;
                # the DVE tail is deferred to the next batch
                pst = psA.tile([P, 16 * T], F32, name="smT", tag="smT")
                for til in range(T):
                    ex = ex_tiles[til // 4]
                    tt = til % 4
                    nc.tensor.transpose(pst[:, 16 * til:16 * til + 10],
                                        ex[:, 128 * tt:128 * (tt + 1)],
                                        idn_t[0:10, 0:10])
                pending = (ib, pst)

            if pending is not None:
                softmax_tail(*pending)
                pending = None

    _split_excess_waits(nc)
    return nc


# ------------------------------------------------------------- numpy driver
def _prep_weights(W1, b1, W2, b2, W3, b3, W4, b4):
    """Fold per-feature affine corrections into W1/b1; transpose for PE."""
    f64 = np.float64
    W1_eff = W1.astype(f64).copy()
    b1_eff = b1.astype(f64).copy()
    for bi, name in enumerate(BLOCKS):
        cols = slice(bi * H, (bi + 1) * H)
        if name in ("mean_1", "mean_2"):
            W1_eff[:, cols] /= H
    return {
        "w1": np.ascontiguousarray(W1_eff.T.astype(np.float32)),
        "b1": b1_eff.astype(np.float32).reshape(-1, 1),
        "w2": np.ascontiguousarray(W2.T.astype(np.float32)),
        "b2": b2.reshape(-1, 1).astype(np.float32),
        "w3": np.ascontiguousarray(W3.T.astype(np.float32)),
        "b3": b3.reshape(-1, 1).astype(np.float32),
        "w4": np.ascontiguousarray(W4.T.astype(np.float32)),
        "b4": b4.reshape(-1, 1).astype(np.float32),
        "idn": np.eye(P, dtype=np.float32),
    }


_NC_CACHE = {}


def _get_nc(n_tiles, **kw):
    key = (n_tiles, tuple(sorted(kw.items())))
    if key not in _NC_CACHE:
        _NC_CACHE[key] = build_nc(n_tiles, **kw)
    return _NC_CACHE[key]


def run(t, weights, n_tiles=N_TILES, trace=False, **kw):
    nc = _get_nc(n_tiles, **kw)
    rows = P * n_tiles
    in_maps = []
    for c in range(N_CORES):
        m = {"t": np.ascontiguousarray(t[c * B_CORE:c * B_CORE + rows])}
        m.update(weights)
        in_maps.append(m)
    res = run_bass_kernel_spmd(nc, in_maps, core_ids=list(range(N_CORES)),
                               trace=trace)
    outs = [r["y"] for r in res.results]
    return outs, res


def kernel(t, W1, b1, W2, b2, W3, b3, W4, b4):
    weights = _prep_weights(W1, b1, W2, b2, W3, b3, W4, b4)
    outs, _ = run(t, weights)
    y = np.concatenate([o[:, 0:10] for o in outs], axis=0)
    return np.ascontiguousarray(y.astype(np.float32))
